# revision 8
# baseline (speedup 1.0000x reference)
"""Trainium2 Bass kernel: int8 GEMM with per-row dequant/requant.

Problem (hardcoded):
    x        [8192, 4096] int8
    weight_q [4096, 4096] int8   (out_features x in_features)
    scale_x  [8192] f32, scale_w [4096] f32, scale_y [8192] f32
    acc[s,n] = sum_k x[s,k] * w[n,k]            (int32 exact)
    out      = clip(round(acc_f32 * ((sx/sy)[:,None] * sw[None,:])), -128, 127).int8
    returns (out_q, scale_y)

Strategy: shard the sequence dim S across 8 cores (1024 rows each); weights
replicated.  Each core computes its [1024, 4096] output slab; host concats.

The PE array has no int8 mode, so the GEMM runs in bf16 (ints up to +-127 are
exact in bf16; products exact; fp32 PSUM accumulation exact for this data).
x is DMA-transposed on chip (HW xbar, 2-byte dtype) to get K on partitions.
Requant runs on DVE: (sw * sxy) * acc in the reference's association order,
clip, then round-to-nearest-even via the +-1.5*2^23 magic-number trick, and
cast to int8.
"""

import numpy as np
import ml_dtypes

S, K, N = 8192, 4096, 4096
NCORES = 8
P = 128
MAGIC = 12582912.0  # 1.5 * 2**23: fp32 RNE round-to-integer trick


def build_nc(s_shard=S // NCORES, k=K, n=N, n_blk=512, repeat=1):
    import concourse.mybir as mybir
    import concourse.tile as tile
    from concourse import bacc

    kt = k // P          # number of 128-row k tiles
    s_tiles = s_shard // P
    n_blks = n // n_blk

    nc = bacc.Bacc("TRN2", target_bir_lowering=False, debug=False,
                   num_devices=NCORES)
    f32 = mybir.dt.float32
    bf16 = mybir.dt.bfloat16
    i8 = mybir.dt.int8
    mult = mybir.AluOpType.mult

    x_d = nc.dram_tensor("xs", [s_shard, k], bf16, kind="ExternalInput")
    w_d = nc.dram_tensor("wt", [k, n], bf16, kind="ExternalInput")
    sw_d = nc.dram_tensor("swb", [P, n], f32, kind="ExternalInput")
    sxy_d = nc.dram_tensor("sxy", [P, s_tiles], f32, kind="ExternalInput")
    o_d = nc.dram_tensor("oq", [s_shard, n], i8, kind="ExternalOutput")

    with tile.TileContext(nc) as tc:
        with (
            tc.tile_pool(name="xT", bufs=1) as xpool,
            tc.tile_pool(name="w", bufs=2) as wpool,
            tc.tile_pool(name="consts", bufs=1) as cpool,
            tc.tile_pool(name="tmp", bufs=4) as tpool,
            tc.tile_pool(name="oq", bufs=4) as qpool,
            tc.tile_pool(name="ps", bufs=8, space="PSUM") as ppool,
        ):
            # DMA queue split: transposes own the sync (SP) HWDGE queue
            # exclusively; weight/scale loads go on the scalar (ACT) HWDGE
            # queue; output stores on gpsimd (SWDGE).
            sxy_t = cpool.tile([P, s_tiles], f32)
            nc.scalar.dma_start(sxy_t[:], sxy_d.ap())

            w_r = w_d.ap().rearrange("(ko p) n -> p ko n", p=P)

            sw_t = cpool.tile([P, n], f32)
            sw_loaded = False

            for rep in range(repeat):
                # first n-block's weights split into chunks so the first
                # matmuls aren't gated on the whole 4MB transfer
                w0_t = wpool.tile([P, kt, n_blk], bf16, tag="w")
                w0_chunk = kt // 4
                for c in range(4):
                    ks = slice(c * w0_chunk, (c + 1) * w0_chunk)
                    nc.scalar.dma_start(w0_t[:, ks, :], w_r[:, ks, 0:n_blk])

                # x transposed to [k_inner(part), k_outer, s]
                xT = xpool.tile([P, kt, s_shard], bf16, tag="xT")
                for kk in range(kt):
                    nc.sync.dma_start_transpose(
                        xT[:, kk, :], x_d.ap()[:, kk * P:(kk + 1) * P])

                if not sw_loaded:
                    nc.scalar.dma_start(sw_t[:], sw_d.ap())
                    sw_loaded = True

                for nb in range(n_blks):
                    if nb == 0:
                        w_t = w0_t
                    else:
                        w_t = wpool.tile([P, kt, n_blk], bf16, tag="w")
                        nc.scalar.dma_start(
                            w_t[:], w_r[:, :, nb * n_blk:(nb + 1) * n_blk])
                    for st in range(s_tiles):
                        ps = ppool.tile([P, n_blk], f32, tag="ps")
                        for kk in range(kt):
                            nc.tensor.matmul(
                                ps[:],
                                lhsT=xT[:, kk, st * P:(st + 1) * P],
                                rhs=w_t[:, kk, :],
                                start=(kk == 0),
                                stop=(kk == kt - 1),
                            )
                        r = tpool.tile([P, n_blk], f32, tag="r")
                        # r = (sw * sxy) * acc  -- reference association order
                        nc.vector.scalar_tensor_tensor(
                            r[:], sw_t[:, nb * n_blk:(nb + 1) * n_blk],
                            sxy_t[:, st:st + 1], ps[:], mult, mult)
                        # clip to [-128, 127] (same result as round-then-clip
                        # because the bounds are integers)
                        nc.vector.tensor_scalar(
                            r[:], r[:], 127.0, -128.0,
                            mybir.AluOpType.min, mybir.AluOpType.max)
                        # round-to-nearest-even + cast to int8
                        q = qpool.tile([P, n_blk], i8, tag="q")
                        nc.vector.tensor_scalar(
                            q[:], r[:], MAGIC, MAGIC,
                            mybir.AluOpType.add, mybir.AluOpType.subtract)
                        nc.gpsimd.dma_start(
                            o_d.ap()[st * P:(st + 1) * P,
                                     nb * n_blk:(nb + 1) * n_blk], q[:])
    nc.compile()
    return nc


def make_in_maps(x, weight_q, scale_x, scale_w, scale_y, ncores=NCORES):
    """Shard + marshal full inputs into per-core input maps."""
    s, k = x.shape
    n = weight_q.shape[0]
    s_shard = s // ncores
    s_tiles = s_shard // P

    x_bf = x.astype(ml_dtypes.bfloat16)
    wT_bf = np.ascontiguousarray(weight_q.T).astype(ml_dtypes.bfloat16)
    sxy = scale_x.astype(np.float32) / scale_y.astype(np.float32)
    sw_b = np.ascontiguousarray(
        np.broadcast_to(scale_w.astype(np.float32), (P, n)))

    in_maps = []
    for c in range(ncores):
        sl = slice(c * s_shard, (c + 1) * s_shard)
        in_maps.append({
            "xs": np.ascontiguousarray(x_bf[sl]),
            "wt": wT_bf,
            "swb": sw_b,
            "sxy": np.ascontiguousarray(sxy[sl].reshape(s_tiles, P).T),
        })
    return in_maps


_NC_CACHE = {}


def _get_nc():
    if "nc" not in _NC_CACHE:
        _NC_CACHE["nc"] = build_nc()
    return _NC_CACHE["nc"]


def kernel(x, weight_q, scale_x, scale_w, scale_y):
    from concourse.bass_utils import run_bass_kernel_spmd

    x = np.asarray(x)
    weight_q = np.asarray(weight_q)
    scale_x = np.asarray(scale_x)
    scale_w = np.asarray(scale_w)
    scale_y = np.asarray(scale_y)

    nc = _get_nc()
    in_maps = make_in_maps(x, weight_q, scale_x, scale_w, scale_y)
    res = run_bass_kernel_spmd(nc, in_maps, core_ids=list(range(NCORES)))
    out = np.concatenate([r["oq"] for r in res.results], axis=0)
    return out, scale_y


# revision 13
# speedup vs baseline: 1.1134x; 1.1134x over previous
"""Trainium2 Bass kernel: int8 GEMM with per-row dequant/requant.

Problem (hardcoded):
    x        [8192, 4096] int8
    weight_q [4096, 4096] int8   (out_features x in_features)
    scale_x  [8192] f32, scale_w [4096] f32, scale_y [8192] f32
    acc[s,n] = sum_k x[s,k] * w[n,k]            (int32 exact)
    out      = clip(round(acc_f32 * ((sx/sy)[:,None] * sw[None,:])), -128, 127).int8
    returns (out_q, scale_y)

Strategy: shard the sequence dim S across 8 cores (1024 rows each); weights
replicated.  Each core computes its [1024, 4096] output slab; host concats.

This Bass/walrus stack exposes no int8 matmul, so the GEMM runs in bf16
(ints up to +-127 are exact in bf16; products exact; fp32 PSUM accumulation
exact for this data -> output matches the int32 reference bit-for-bit).
The host marshals x k-major (x_pret) so K lands on SBUF partitions with plain
contiguous DMAs; build_nc(x_pret=False) keeps the on-chip HW xbar
dma_start_transpose path (~30us slower end-to-end).
Requant runs on DVE: (sw * sxy) * acc in the reference's association order,
clip, then round-to-nearest-even via the +-1.5*2^23 magic-number trick, and
cast to int8.

Measured (8 cores, interleaved repeat-slope method): ~468 us vs a bf16 PE
roofline of ~437 us (2048 N=512 matmuls/core at 1 MAC/cell/cycle, 2.4 GHz).
"""

import numpy as np
import ml_dtypes

S, K, N = 8192, 4096, 4096
NCORES = 8
P = 128
MAGIC = 12582912.0  # 1.5 * 2**23: fp32 RNE round-to-integer trick


def build_nc(s_shard=S // NCORES, k=K, n=N, n_blk=512, repeat=1,
             x_pret=False):
    import concourse.mybir as mybir
    import concourse.tile as tile
    from concourse import bacc

    kt = k // P          # number of 128-row k tiles
    s_tiles = s_shard // P
    n_blks = n // n_blk

    nc = bacc.Bacc("TRN2", target_bir_lowering=False, debug=False,
                   num_devices=NCORES)
    f32 = mybir.dt.float32
    bf16 = mybir.dt.bfloat16
    i8 = mybir.dt.int8
    mult = mybir.AluOpType.mult

    x_shape = [k, s_shard] if x_pret else [s_shard, k]
    x_d = nc.dram_tensor("xs", x_shape, bf16, kind="ExternalInput")
    w_d = nc.dram_tensor("wt", [k, n], bf16, kind="ExternalInput")
    sw_d = nc.dram_tensor("swb", [P, n], f32, kind="ExternalInput")
    sxy_d = nc.dram_tensor("sxy", [P, s_tiles], f32, kind="ExternalInput")
    o_d = nc.dram_tensor("oq", [s_shard, n], i8, kind="ExternalOutput")

    with tile.TileContext(nc) as tc:
        with (
            tc.tile_pool(name="xT", bufs=1) as xpool,
            tc.tile_pool(name="w", bufs=2) as wpool,
            tc.tile_pool(name="consts", bufs=1) as cpool,
            tc.tile_pool(name="tmp", bufs=4) as tpool,
            tc.tile_pool(name="oq", bufs=4) as qpool,
            tc.tile_pool(name="ps", bufs=8, space="PSUM") as ppool,
        ):
            # DMA queue split: transposes own the sync (SP) HWDGE queue
            # exclusively; weight/scale loads go on the scalar (ACT) HWDGE
            # queue; output stores on gpsimd (SWDGE).
            sxy_t = cpool.tile([P, s_tiles], f32)
            nc.scalar.dma_start(sxy_t[:], sxy_d.ap())

            w_r = w_d.ap().rearrange("(ko p) n -> p ko n", p=P)

            sw_t = cpool.tile([P, n], f32)
            sw_loaded = False

            for rep in range(repeat):
                # first n-block's weights split into chunks so the first
                # matmuls aren't gated on the whole 4MB transfer
                w0_t = wpool.tile([P, kt, n_blk], bf16, tag="w")
                w0_chunk = kt // 4
                for c in range(4):
                    ks = slice(c * w0_chunk, (c + 1) * w0_chunk)
                    nc.scalar.dma_start(w0_t[:, ks, :], w_r[:, ks, 0:n_blk])

                # x transposed to [k_inner(part), k_outer, s]
                xT = xpool.tile([P, kt, s_shard], bf16, tag="xT")
                if x_pret:
                    xr = x_d.ap().rearrange("(ko p) s -> p ko s", p=P)
                    for kk in range(kt):
                        nc.sync.dma_start(xT[:, kk, :], xr[:, kk, :])
                else:
                    for kk in range(kt):
                        nc.sync.dma_start_transpose(
                            xT[:, kk, :], x_d.ap()[:, kk * P:(kk + 1) * P])

                if not sw_loaded:
                    nc.scalar.dma_start(sw_t[:], sw_d.ap())
                    sw_loaded = True

                for nb in range(n_blks):
                    if nb == 0:
                        w_t = w0_t
                    else:
                        w_t = wpool.tile([P, kt, n_blk], bf16, tag="w")
                        nc.scalar.dma_start(
                            w_t[:], w_r[:, :, nb * n_blk:(nb + 1) * n_blk])
                    for st in range(s_tiles):
                        ps = ppool.tile([P, n_blk], f32, tag="ps")
                        for kk in range(kt):
                            nc.tensor.matmul(
                                ps[:],
                                lhsT=xT[:, kk, st * P:(st + 1) * P],
                                rhs=w_t[:, kk, :],
                                start=(kk == 0),
                                stop=(kk == kt - 1),
                            )
                        r = tpool.tile([P, n_blk], f32, tag="r")
                        # r = (sw * sxy) * acc  -- reference association order
                        nc.vector.scalar_tensor_tensor(
                            r[:], sw_t[:, nb * n_blk:(nb + 1) * n_blk],
                            sxy_t[:, st:st + 1], ps[:], mult, mult)
                        # clip to [-128, 127] (same result as round-then-clip
                        # because the bounds are integers)
                        nc.vector.tensor_scalar(
                            r[:], r[:], 127.0, -128.0,
                            mybir.AluOpType.min, mybir.AluOpType.max)
                        # round-to-nearest-even + cast to int8
                        q = qpool.tile([P, n_blk], i8, tag="q")
                        nc.vector.tensor_scalar(
                            q[:], r[:], MAGIC, MAGIC,
                            mybir.AluOpType.add, mybir.AluOpType.subtract)
                        nc.gpsimd.dma_start(
                            o_d.ap()[st * P:(st + 1) * P,
                                     nb * n_blk:(nb + 1) * n_blk], q[:])
    nc.compile()
    return nc


def make_in_maps(x, weight_q, scale_x, scale_w, scale_y, ncores=NCORES,
                 x_pret=False):
    """Shard + marshal full inputs into per-core input maps."""
    s, k = x.shape
    n = weight_q.shape[0]
    s_shard = s // ncores
    s_tiles = s_shard // P

    x_bf = x.astype(ml_dtypes.bfloat16)
    if x_pret:
        x_bf = np.ascontiguousarray(x_bf.T)  # [K, S]
    wT_bf = np.ascontiguousarray(weight_q.T).astype(ml_dtypes.bfloat16)
    sxy = scale_x.astype(np.float32) / scale_y.astype(np.float32)
    sw_b = np.ascontiguousarray(
        np.broadcast_to(scale_w.astype(np.float32), (P, n)))

    in_maps = []
    for c in range(ncores):
        sl = slice(c * s_shard, (c + 1) * s_shard)
        in_maps.append({
            "xs": np.ascontiguousarray(x_bf[:, sl] if x_pret else x_bf[sl]),
            "wt": wT_bf,
            "swb": sw_b,
            "sxy": np.ascontiguousarray(sxy[sl].reshape(s_tiles, P).T),
        })
    return in_maps


_NC_CACHE = {}


def _get_nc():
    if "nc" not in _NC_CACHE:
        _NC_CACHE["nc"] = build_nc(x_pret=True)
    return _NC_CACHE["nc"]


def kernel(x, weight_q, scale_x, scale_w, scale_y):
    from concourse.bass_utils import run_bass_kernel_spmd

    x = np.asarray(x)
    weight_q = np.asarray(weight_q)
    scale_x = np.asarray(scale_x)
    scale_w = np.asarray(scale_w)
    scale_y = np.asarray(scale_y)

    nc = _get_nc()
    in_maps = make_in_maps(x, weight_q, scale_x, scale_w, scale_y, x_pret=True)
    res = run_bass_kernel_spmd(nc, in_maps, core_ids=list(range(NCORES)))
    out = np.concatenate([r["oq"] for r in res.results], axis=0)
    return out, scale_y
